# revision 27
# baseline (speedup 1.0000x reference)
"""DGI (3-layer GCN encoder x2 + bilinear discriminator) Trainium2 Bass kernel.

Strategy (8 NeuronCores, 1D row-parallel over nodes):
  - Each core owns a 2048-row block of the 16384-node graph.
  - Layer 1 uses associativity: adj @ (X W1) == (adj @ X) W1, so the big
    n^2 matmul runs at width 64 per encoder instead of 264 (4.1x less PE
    work), followed by a tiny [64 x 264] projection fused into the drain.
  - adj is stored twice, pre-transposed + tiled on the host:
      A8  = e4m3(adj*n - 0.5)  (fp8, mean-centered)  -> layers 1 and 3
      A16 = fp16(adj*n)                               -> layer 2
    fp8 halves the HBM traffic of the DMA-bound layers 1/3; centering at
    the distribution mean keeps the quantization noise ~5x below raw fp8
    (verified numerically: matches the fp16-only error). The 0.5-mean term
    is restored exactly via column sums of the moving operand: for L1 it
    is folded into the bias on the host; for L3 a 0.5-weighted
    ones-matmul over p3 computes it on device (~3us).
  - Layers 1/3 run "flipped" (adj slab is the moving operand, activations
    stationary) so the PE emits transposed outputs directly, avoiding
    per-row-chunk PE transposes. Layer 2 (width 328) runs in normal
    orientation (psum [rows, 328]) which is cheaper than flipping.
  - Activations p = full [n, d] fp16 tensors in SBUF, both encoders packed
    side by side. After each layer, the local block is projected by the
    next W and AllGather'ed across the 8 cores in two row-halves that
    overlap compute.
  - Readout: node-sum of h3 (enc1) via DVE free-dim reduce on h3T,
    AllGather + local reduce, sigmoid -> c; cw = wd @ c; scores via PE
    matvec on h3T.
"""

import sys
import time

import numpy as np

sys.path.insert(0, "/opt/trn_rl_repo")

import concourse.bass as bass  # noqa: E402
import concourse.mybir as mybir  # noqa: E402
import concourse.tile as tile  # noqa: E402
from concourse import bacc  # noqa: E402

P = 128
NCORES = 8
ADJ_FRAC = 1  # diagnostic: stride over adj slabs (1 = all, 2 = half, ...)
D0, D1, D2, D3 = 64, 264, 164, 64
NS1, NS2 = 3, 2  # 128-subtiles of the (padded) dims 264 -> 3, 164 -> 2
SCALE = 16384.0
KO = 4  # k-tiles per slab DMA
GSZ = 4  # row-chunks per m-group
DT8 = mybir.dt.float8e4
DT16 = mybir.dt.float16
DT32 = mybir.dt.float32
AF = mybir.ActivationFunctionType
ALU = mybir.AluOpType


def _params(n):
    R = n // NCORES
    return dict(
        R=R,
        RC=R // P,
        KT=n // P,
        NG=(R // P) // GSZ,
        KB=(n // P) // KO,
        CH=max(1, R // (2 * P)),  # k-tiles per resident p chunk (half a rank)
        SCW=min(512, R),  # score output chunk
    )


def build_program(n=16384):
    pr = _params(n)
    R, RC, KT, NG, KB, CH, SCW = (
        pr["R"],
        pr["RC"],
        pr["KT"],
        pr["NG"],
        pr["KB"],
        pr["CH"],
        pr["SCW"],
    )
    NPC = KT // CH
    W = GSZ * P
    RH = R // 2
    RC2 = RC // 2

    nc = bacc.Bacc(
        "TRN2", target_bir_lowering=False, debug=False, num_devices=NCORES
    )

    # adjT pre-tiled on host: [NG, KB, P, KO, W]; each slab DMA is a fully
    # contiguous block (2KB-per-partition descriptors for both dtypes).
    A8 = nc.dram_tensor("adjT8", [NG, KB, P, KO, W], DT8, kind="ExternalInput").ap()
    SQX = nc.dram_tensor("seqx", [P, KT, 2 * D0], DT16, kind="ExternalInput").ap()
    W1 = nc.dram_tensor("w1", [P, D1], DT16, kind="ExternalInput").ap()
    W2 = nc.dram_tensor("w2", [NS1 * P, D2], DT16, kind="ExternalInput").ap()
    W3 = nc.dram_tensor("w3", [NS2 * P, D3], DT16, kind="ExternalInput").ap()
    B1C = nc.dram_tensor("b1c", [P, 2 * NS1], DT32, kind="ExternalInput").ap()
    B2 = nc.dram_tensor("b2", [P, 2 * D2], DT16, kind="ExternalInput").ap()
    B3 = nc.dram_tensor("b3", [P, 1], DT32, kind="ExternalInput").ap()
    WDT = nc.dram_tensor("wdt", [P, D3], DT32, kind="ExternalInput").ap()
    IDT = nc.dram_tensor("ident", [P, P], DT16, kind="ExternalInput").ap()
    SB = nc.dram_tensor("sb", [1, 2 * R], DT32, kind="ExternalInput").ap()
    OUT = nc.dram_tensor("out", [2, R], DT32, kind="ExternalOutput").ap()

    rg = [list(range(NCORES))]

    with tile.TileContext(nc) as tc:
        with (
            tc.tile_pool(name="const", bufs=1) as cp,
            tc.tile_pool(name="p2", bufs=NPC) as pp2,
            tc.tile_pool(name="p3", bufs=NPC) as pp3,
            tc.tile_pool(name="slab8", bufs=5) as s8p,
            tc.tile_pool(name="q1", bufs=2) as q1p,
            tc.tile_pool(name="h", bufs=4) as hp,
            tc.tile_pool(name="hT", bufs=2) as htp,
            tc.tile_pool(name="misc", bufs=4) as mp_,
            tc.tile_pool(name="sc", bufs=4) as scp_,
            tc.tile_pool(name="ploc", bufs=1) as plp,
            tc.tile_pool(name="ps", bufs=8, space="PSUM") as ps,
            tc.tile_pool(name="dram", bufs=1, space="DRAM") as dram,
        ):
            # ---- constants -> SBUF
            w1t = cp.tile([P, D1], DT16, name="w1t")
            nc.sync.dma_start(w1t[:], W1[:])
            w2t = cp.tile([P, NS1, D2], DT16, name="w2t")
            nc.sync.dma_start(w2t[:], W2.rearrange("(s p) d -> p s d", p=P))
            w3t = cp.tile([P, NS2, D3], DT16, name="w3t")
            nc.sync.dma_start(w3t[:], W3.rearrange("(s p) d -> p s d", p=P))
            b1c = cp.tile([P, 2 * NS1], DT32, name="b1c")
            nc.sync.dma_start(b1c[:], B1C[:])
            b2t = cp.tile([P, 2 * D2], DT16, name="b2t")
            nc.sync.dma_start(b2t[:], B2[:])
            b3t = cp.tile([P, 1], DT32, name="b3t")
            nc.sync.dma_start(b3t[:], B3[:])
            wdtt = cp.tile([P, D3], DT32, name="wdtt")
            nc.sync.dma_start(wdtt[:], WDT[:])
            idt = cp.tile([P, P], DT16, name="idt")
            nc.sync.dma_start(idt[:], IDT[:])
            halfs = cp.tile([P, 1], DT16, name="halfs")
            nc.vector.memset(halfs[:], 0.5)
            onesrow = cp.tile([1, P], DT16, name="onesrow")
            nc.vector.memset(onesrow[:], 1.0)
            # X (both encoders side by side), resident; chunked DMAs so L1
            # can start after the first chunk lands
            seqx = cp.tile([P, KT, 2 * D0], DT16, name="seqx")
            NXC = 8
            for s in range(NXC):
                lo = s * (KT // NXC)
                hi = (s + 1) * (KT // NXC)
                nc.gpsimd.dma_start(seqx[:, lo:hi, :], SQX[:, lo:hi, :])

            # ---- DRAM bounce buffers for collectives (split in row-halves
            # so each AllGather overlaps the next compute phase)
            p2l = [dram.tile([RH, 2 * D2], DT16, name=f"p2l{h}") for h in range(2)]
            p2f = [
                dram.tile([n // 2, 2 * D2], DT16, name=f"p2f{h}", addr_space="Shared")
                for h in range(2)
            ]
            p3l = [dram.tile([RH, 2 * D3], DT16, name=f"p3l{h}") for h in range(2)]
            p3f = [
                dram.tile([n // 2, 2 * D3], DT16, name=f"p3f{h}", addr_space="Shared")
                for h in range(2)
            ]
            ssi = dram.tile([64, 1], DT32, name="ssi")
            ssg = dram.tile([64 * NCORES, 1], DT32, name="ssg", addr_space="Shared")
            s2i = dram.tile([1, 2 * D2], DT32, name="s2i")
            s2g = dram.tile([1, 2 * D2], DT32, name="s2g", addr_space="Shared")

            # kb visit order for L2/L3: first-half chunks (even) before
            # second-half, so a layer can start while the second AllGather
            # half is in flight. Valid because KO <= CH.
            if KO <= CH:
                kb_order = [j for j in range(KB) if ((j * KO) // CH) % 2 == 0]
                kb_order += [j for j in range(KB) if ((j * KO) // CH) % 2 == 1]
            else:
                kb_order = list(range(KB))

            # ---- p staging: p_next = h @ W (local rows), per row-half; the
            # first half's matmuls + AllGather are issued mid-layer.
            # colsum_ps: optional [1, 2*d_next] psum accumulating
            # 0.5 * sum over this half's local rows of p_next (for the fp8
            # centering correction of the next layer).
            def p_stage_half(
                hT, wt, ns, d_next, ploc_bufs, pf_bufs, tagix, h, colsum_ps=None
            ):
                ploc = plp.tile(
                    [P, RC2, 2 * d_next], DT16, tag="ploc", name=f"pl{tagix}_{h}"
                )
                for rcl in range(RC2):
                    rc = h * RC2 + rcl
                    for e in range(2):
                        pq = ps.tile(
                            [P, d_next], DT32, tag="ps", name=f"pq{tagix}_{e}_{rc}"
                        )
                        for ds in range(ns):
                            nc.tensor.matmul(
                                pq[:],
                                hT[e][:, ds, rc * P : (rc + 1) * P],
                                wt[:, ds, :],
                                start=(ds == 0),
                                stop=(ds == ns - 1),
                            )
                        nc.scalar.mul(
                            ploc[:, rcl, e * d_next : (e + 1) * d_next],
                            pq[:],
                            1.0 / SCALE,
                        )
                if colsum_ps is not None:
                    csp = ps.tile(
                        [1, 2 * d_next], DT32, tag="ps", name=f"cs{tagix}_{h}"
                    )
                    for rcl in range(RC2):
                        nc.tensor.matmul(
                            csp[:],
                            halfs[:],
                            ploc[:, rcl, :],
                            start=(rcl == 0),
                            stop=(rcl == RC2 - 1),
                        )
                    # drain to SBUF immediately so the bank recycles
                    nc.vector.tensor_copy(colsum_ps[:], csp[:])
                nc.sync.dma_start(
                    ploc_bufs[h][:].rearrange("(rc p) d -> p rc d", p=P), ploc[:]
                )
                nc.gpsimd.collective_compute(
                    "AllGather",
                    ALU.bypass,
                    replica_groups=rg,
                    ins=[ploc_bufs[h].opt()],
                    outs=[pf_bufs[h].opt()],
                )

            def make_pnext(pool, d_next, tagix):
                # chunk c covers k-tiles [c*CH, (c+1)*CH) = rank c//2, half c%2
                return [
                    pool.tile(
                        [P, CH, 2 * d_next], DT16, tag="p", name=f"p{tagix}c{c}"
                    )
                    for c in range(NPC)
                ]

            def p_loads_half(newp, pf_bufs, h):
                # gpsimd (SWDGE) ring so a slot-wait here never stalls the
                # sync/scalar rings that stream adjT slabs
                RH_ = CH * P  # rows per (rank, half)
                for c in range(h, NPC, 2):
                    rank = c // 2
                    nc.gpsimd.dma_start(
                        newp[c][:],
                        pf_bufs[h][:][rank * RH_ : (rank + 1) * RH_, :].rearrange(
                            "(ko p) d -> p ko d", p=P
                        ),
                    )

            # =========== Layer 1 (flipped, fp8-centered adj moving) =========
            # psum[j, r] = sum_k X[k, j] * adjc[k, r]  -> n*(adj@X).T - 0.5*s1
            # drain: h1T[ds] = relu(W1[:,ds].T @ q1T + b1c)  (b1c holds the
            # +0.5*W1.T@colsum(X) correction + n*b1, folded on the host)
            hT1 = [
                htp.tile([P, NS1, R], DT16, tag="hT", name=f"h1T{e}") for e in range(2)
            ]
            for e in range(2):
                nc.vector.memset(hT1[e][:, NS1 - 1, :], 0.0)

            p2c = make_pnext(pp2, D2, 2)
            s2hs = [
                mp_.tile([1, 2 * D2], DT32, tag="s2", name=f"s2h{h}")
                for h in range(2)
            ]

            def stage2a():
                p_stage_half(hT1, w2t, NS1, D2, p2l, p2f, 2, 0, colsum_ps=s2hs[0])
                p_loads_half(p2c, p2f, 0)

            kbs1 = list(range(KB))[::ADJ_FRAC]
            for g in range(NG):
                ps1 = ps.tile([P, W], DT32, tag="ps", name=f"q1_{g}")
                for ki, kb in enumerate(kbs1):
                    slab = s8p.tile([P, KO, W], DT8, tag="slab8", name=f"sl1_{g}_{kb}")
                    eng = nc.sync if ki % 2 == 0 else nc.scalar
                    eng.dma_start(slab[:], A8[g, kb])
                    for ko in range(KO):
                        kt = kb * KO + ko
                        nc.tensor.matmul(
                            ps1[:],
                            seqx[:, kt, :],
                            slab[:, ko, :],
                            start=(ki == 0 and ko == 0),
                            stop=(ki == len(kbs1) - 1 and ko == KO - 1),
                        )
                # drain group g
                q1sb = q1p.tile([P, W], DT16, tag="q1", name=f"q1sb_{g}")
                nc.vector.tensor_copy(q1sb[:], ps1[:])
                for e in range(2):
                    for ds in range(NS1):
                        cs = min(P, D1 - ds * P)
                        pA = ps.tile([cs, W], DT32, tag="ps", name=f"hA{g}_{e}_{ds}")
                        nc.tensor.matmul(
                            pA[:],
                            w1t[e * 64 : (e + 1) * 64, ds * P : ds * P + cs],
                            q1sb[e * 64 : (e + 1) * 64, :],
                            start=True,
                            stop=True,
                        )
                        nc.scalar.activation(
                            hT1[e][0:cs, ds, g * W : (g + 1) * W],
                            pA[:],
                            AF.Relu,
                            bias=b1c[0:cs, e * NS1 + ds : e * NS1 + ds + 1],
                        )
                if g == NG // 2 - 1:
                    stage2a()

            p_stage_half(hT1, w2t, NS1, D2, p2l, p2f, 2, 1, colsum_ps=s2hs[1])
            p_loads_half(p2c, p2f, 1)
            # combine local colsum halves and AllReduce across cores; the
            # result is consumed ~70us later (L2 group 0 drain), hiding the
            # collective latency under the first group's accumulation.
            s2sb = mp_.tile([1, 2 * D2], DT32, tag="s2", name="s2sb")
            nc.vector.tensor_tensor(s2sb[:], s2hs[0][:], s2hs[1][:], ALU.add)
            nc.scalar.dma_start(s2i[:], s2sb[:])
            nc.gpsimd.collective_compute(
                "AllReduce",
                ALU.add,
                replica_groups=rg,
                ins=[s2i.opt()],
                outs=[s2g.opt()],
            )

            # =========== Layer 2 (normal, fp8-centered adj stationary) ======
            # q2[r, j] = psum + 0.5*colsum(p2)[j]; the correction (at scale
            # S^0, since adj_s = 0.5 + adj_c at scale S) plus n*b2 forms the
            # broadcast bias tile C2 applied in the drain.
            hT2 = [
                htp.tile([P, NS2, R], DT16, tag="hT", name=f"h2T{e}") for e in range(2)
            ]
            for e in range(2):
                nc.vector.memset(hT2[e][:, NS2 - 1, :], 0.0)

            c2t = cp.tile([P, 2 * D2], DT16, name="c2t")

            def post2(rc, q2):
                r0 = rc * P
                h = hp.tile([P, 2 * D2], DT16, tag="h", name=f"h2_{rc}")
                nc.vector.tensor_tensor(h[:], q2[:], c2t[:], ALU.add)
                nc.scalar.activation(h[:], h[:], AF.Relu)
                for e in range(2):
                    for ds in range(NS2):
                        lo = e * D2 + ds * P
                        csz = min(P, D2 - ds * P)
                        tp = ps.tile([csz, P], DT16, tag="ps", name=f"t2_{rc}_{e}_{ds}")
                        nc.tensor.transpose(tp[:], h[:, lo : lo + csz], idt[:])
                        nc.vector.tensor_copy(hT2[e][0:csz, ds, r0 : r0 + P], tp[:])

            p3c = make_pnext(pp3, D3, 3)

            def stage3a():
                p_stage_half(hT2, w3t, NS2, D3, p3l, p3f, 3, 0)
                p_loads_half(p3c, p3f, 0)

            for g in range(NG):
                q2s = [
                    ps.tile([P, 2 * D2], DT32, tag="ps", name=f"q2_{g}_{mc}")
                    for mc in range(GSZ)
                ]
                kbs2 = kb_order[::ADJ_FRAC]
                for ki, kb in enumerate(kbs2):
                    slab = s8p.tile([P, KO, W], DT8, tag="slab8", name=f"sl2_{g}_{kb}")
                    eng = nc.sync if ki % 2 == 0 else nc.scalar
                    eng.dma_start(slab[:], A8[g, kb])
                    for ko in range(KO):
                        k = kb * KO + ko
                        rhs_t = p2c[k // CH]
                        for mc in range(GSZ):
                            nc.tensor.matmul(
                                q2s[mc][:],
                                slab[:, ko, mc * P : (mc + 1) * P],
                                rhs_t[:, k % CH, :],
                                start=(ki == 0 and ko == 0),
                                stop=(ki == len(kbs2) - 1 and ko == KO - 1),
                            )
                if g == 0:
                    # build C2 = bcast(0.5*colsum(p2)) + n*b2 once the
                    # AllReduce result is back (hidden under g0 accumulation)
                    s2r32 = mp_.tile([1, 2 * D2], DT32, tag="s2", name="s2r32")
                    nc.gpsimd.dma_start(s2r32[:], s2g[:])
                    s2r16 = mp_.tile([1, 2 * D2], DT16, tag="s2", name="s2r16")
                    nc.vector.tensor_copy(s2r16[:], s2r32[:])
                    psC2 = ps.tile([P, 2 * D2], DT32, tag="ps", name="psC2")
                    nc.tensor.matmul(
                        psC2[:], onesrow[:], s2r16[:], start=True, stop=True
                    )
                    nc.vector.tensor_tensor(c2t[:], psC2[:], b2t[:], ALU.add)
                for mc in range(GSZ):
                    post2(g * GSZ + mc, q2s[mc])
                if g == NG // 2 - 1:
                    stage3a()

            p_stage_half(hT2, w3t, NS2, D3, p3l, p3f, 3, 1)
            p_loads_half(p3c, p3f, 1)

            # =========== Layer 3 (flipped, fp8-centered adj moving) =========
            # psum[j, r] = sum_k p3[k, j] * adjc[k, r]; bias3 = 0.5*colsum(p3)
            # + n*b3 restores the centering term, fused into the relu drain.
            # The 0.5*colsum runs as ones(0.5)-matmuls over the gathered p3
            # chunks, split in halves so the second half can wait for its
            # AllGather while L3's group 0 accumulates.
            h3T = htp.tile([P, R], DT16, tag="hT", name="h3Tcat")
            s3ps = [ps.tile([P, 1], DT32, tag="ps", name=f"s3_{h}") for h in range(2)]

            def s3_half(h):
                idx = [(c, ch) for c in range(h, NPC, 2) for ch in range(CH)]
                for i, (c, ch) in enumerate(idx):
                    nc.tensor.matmul(
                        s3ps[h][:],
                        p3c[c][:, ch, :],
                        halfs[:],
                        start=(i == 0),
                        stop=(i == len(idx) - 1),
                    )

            s3_half(0)
            s3sb = mp_.tile([P, 1], DT32, tag="misc", name="s3sb")
            nc.vector.tensor_copy(s3sb[:], s3ps[0][:])
            bias3 = mp_.tile([P, 1], DT32, tag="misc", name="bias3")

            kbs3 = kb_order[::ADJ_FRAC]
            for g in range(NG):
                q3 = ps.tile([P, W], DT32, tag="ps", name=f"q3_{g}")
                for ki, kb in enumerate(kbs3):
                    slab = s8p.tile([P, KO, W], DT8, tag="slab8", name=f"sl3_{g}_{kb}")
                    eng = (nc.sync, nc.scalar, nc.gpsimd)[ki % 3]
                    eng.dma_start(slab[:], A8[g, kb])
                    for ko in range(KO):
                        k = kb * KO + ko
                        nc.tensor.matmul(
                            q3[:],
                            p3c[k // CH][:, k % CH, :],
                            slab[:, ko, :],
                            start=(ki == 0 and ko == 0),
                            stop=(ki == len(kbs3) - 1 and ko == KO - 1),
                        )
                if g == 0:
                    # second-half colsum + bias once its chunks have landed
                    s3_half(1)
                    nc.vector.tensor_tensor(bias3[:], s3ps[1][:], s3sb[:], ALU.add)
                    nc.vector.tensor_tensor(bias3[:], bias3[:], b3t[:], ALU.add)
                nc.scalar.activation(
                    h3T[:, g * W : (g + 1) * W],
                    q3[:],
                    AF.Relu,
                    bias=bias3[:],
                )

            # ---- readout: c = sigmoid(mean_n h3_enc1); cw = wd @ c; scores
            ss = mp_.tile([P, 1], DT32, tag="misc", name="ss")
            nc.vector.reduce_sum(
                ss[0:64, :], h3T[0:64, :], axis=mybir.AxisListType.X
            )
            nc.sync.dma_start(ssi[:], ss[0:64, :])
            nc.gpsimd.collective_compute(
                "AllGather",
                ALU.bypass,
                replica_groups=rg,
                ins=[ssi.opt()],
                outs=[ssg.opt()],
            )
            cin = mp_.tile([64, NCORES], DT32, tag="misc", name="cin")
            nc.sync.dma_start(
                cin[:], ssg[:].rearrange("(c p) one -> p (c one)", p=64)
            )
            cin2 = mp_.tile([64, 1], DT32, tag="misc", name="cin2")
            nc.vector.reduce_sum(cin2[:], cin[:], axis=mybir.AxisListType.X)
            ccol = mp_.tile([P, 1], DT32, tag="misc", name="ccol")
            nc.vector.memset(ccol[:], 0.0)
            nc.scalar.activation(
                ccol[0:64, :], cin2[:], AF.Sigmoid, scale=1.0 / (SCALE * n)
            )
            cwps = ps.tile([64, 1], DT32, tag="ps", name="cwps")
            nc.tensor.matmul(cwps[:], wdtt[:], ccol[:], start=True, stop=True)
            # two masked copies of cw: cwa selects enc1 partitions, cwb enc2
            cw16 = [
                mp_.tile([P, 1], DT16, tag="misc", name=f"cw16_{e}") for e in range(2)
            ]
            for e in range(2):
                nc.vector.memset(cw16[e][:], 0.0)
                nc.vector.tensor_copy(cw16[e][e * D3 : (e + 1) * D3, :], cwps[:])
            # score epilogue: all matmuls issued back-to-back, per-chunk
            # scale/bias/store pipelined on dedicated pool slots
            for e in range(2):
                for j in range(R // SCW):
                    scp = ps.tile([1, SCW], DT32, tag="ps", name=f"scp{e}_{j}")
                    nc.tensor.matmul(
                        scp[:],
                        cw16[e][:],
                        h3T[:, j * SCW : (j + 1) * SCW],
                        start=True,
                        stop=True,
                    )
                    sbc = scp_.tile([1, SCW], DT32, tag="sc", name=f"sbc{e}_{j}")
                    nc.sync.dma_start(
                        sbc[:], SB[:, e * R + j * SCW : e * R + (j + 1) * SCW]
                    )
                    sct = scp_.tile([1, SCW], DT32, tag="sc", name=f"sct{e}_{j}")
                    nc.scalar.mul(sct[:], scp[:], 1.0 / SCALE)
                    ot = scp_.tile([1, SCW], DT32, tag="sc", name=f"ot{e}_{j}")
                    nc.vector.tensor_tensor(ot[:], sct[:], sbc[:], ALU.add)
                    nc.scalar.dma_start(OUT[e : e + 1, j * SCW : (j + 1) * SCW], ot[:])

    nc.compile()
    return nc


# ---------------------------------------------------------------------------
# host-side input prep


def _blocked_transpose(a, B=512):
    n, m = a.shape
    out = np.empty((m, n), a.dtype)
    for i in range(0, n, B):
        for j in range(0, m, B):
            out[j : j + B, i : i + B] = a[i : i + B, j : j + B].T
    return out


def _tile_adjT(aT, NG, KB, W):
    """[n(k), n(r)] -> [NCORES*NG, KB, P, KO, W] contiguous slabs."""
    n = aT.shape[0]
    R = NG * W
    out = np.empty((NCORES * NG, KB, P, KO, W), aT.dtype)
    for c in range(NCORES):
        blk = np.ascontiguousarray(aT[:, c * R : (c + 1) * R])
        t = blk.reshape(KB, KO, P, NG, W).transpose(3, 0, 2, 1, 4)
        out[c * NG : (c + 1) * NG] = t
    return out


def prep_concat_inputs(inputs, n):
    import ml_dtypes

    R = n // NCORES
    pr = _params(n)
    KT, NG, KB = pr["KT"], pr["NG"], pr["KB"]
    W = GSZ * P

    adj = np.asarray(inputs["adj"], np.float32)[0]
    seq1 = np.asarray(inputs["seq1"], np.float32)[0]
    seq2 = np.asarray(inputs["seq2"], np.float32)[0]
    w1 = np.asarray(inputs["w1"], np.float32)
    w2 = np.asarray(inputs["w2"], np.float32)
    w3 = np.asarray(inputs["w3"], np.float32)
    b1 = np.asarray(inputs["b1"], np.float32)
    b2 = np.asarray(inputs["b2"], np.float32)
    b3 = np.asarray(inputs["b3"], np.float32)
    wd = np.asarray(inputs["wd"], np.float32)
    bd = np.float32(np.asarray(inputs["bd"]))
    sb1 = np.asarray(inputs["samp_bias1"], np.float32)[0]
    sb2 = np.asarray(inputs["samp_bias2"], np.float32)[0]

    S = np.float32(SCALE)

    # fp8 centered adjT (all three layers)
    a8 = (adj * S - np.float32(0.5)).astype(ml_dtypes.float8_e4m3)
    a8T = _blocked_transpose(a8.view(np.uint8)).view(ml_dtypes.float8_e4m3)
    del a8
    adjT8 = _tile_adjT(a8T, NG, KB, W)
    del a8T

    # X (both encoders), [P, KT, 128] layout: [p, kt, e*64+d] = seq_e[kt*P+p, d]
    X16 = np.concatenate([seq1, seq2], axis=1).astype(np.float16)  # [n, 128]
    seqx = np.ascontiguousarray(X16.reshape(KT, P, 2 * D0).transpose(1, 0, 2))

    # b1c: per-partition bias for the L1 drain, including the centering
    # correction 0.5 * W1.T @ colsum(X) (per encoder), at scale n.
    s1 = X16.astype(np.float32).sum(axis=0)  # [128]
    w1r = w1.astype(np.float16).astype(np.float32)  # match device rounding
    b1c = np.zeros((P, 2 * NS1), np.float32)
    for e in range(2):
        corr = 0.5 * (w1r.T @ s1[e * 64 : (e + 1) * 64])  # [264]
        full = b1 * S + corr
        for ds in range(NS1):
            cs = min(P, D1 - ds * P)
            b1c[0:cs, e * NS1 + ds] = full[ds * P : ds * P + cs]

    def padz(a, shape):
        out = np.zeros(shape, np.float16)
        out[: a.shape[0], : a.shape[1]] = a
        return out

    def rep(x):
        return np.tile(np.asarray(x), (NCORES, 1))

    cat = {
        "adjT8": adjT8,
        "seqx": np.tile(seqx.reshape(P, -1), (NCORES, 1)).reshape(
            NCORES * P, KT, 2 * D0
        ),
        # W1 stacked twice along partitions so each encoder's drain matmul
        # reads lhsT and rhs at the same base partition (0 or 64)
        "w1": rep(np.concatenate([w1, w1], axis=0).astype(np.float16)),
        "w2": rep(padz(w2, (NS1 * P, D2))),
        "w3": rep(padz(w3, (NS2 * P, D3))),
        "b1c": rep(b1c),
        "b2": rep(
            np.tile(
                np.concatenate([b2, b2]).astype(np.float32) * S,
                (P, 1),
            ).astype(np.float16)
        ),
        "b3": rep(
            np.concatenate([b3, b3]).astype(np.float32)[:, None] * S
        ),
        "wdt": rep(padz(wd.T, (P, D3)).astype(np.float32)),
        "ident": rep(np.eye(P, dtype=np.float16)),
        "sb": np.concatenate(
            [
                np.concatenate(
                    [sb1[c * R : (c + 1) * R] + bd, sb2[c * R : (c + 1) * R] + bd]
                )[None, :]
                for c in range(NCORES)
            ],
            axis=0,
        ).astype(np.float32),
    }
    return cat


# ---------------------------------------------------------------------------
# cached PJRT executor (compile once, run many)

_EXEC = {}


def make_state(nc):
    """Build a cached shard_map executable for a compiled Bass program."""
    import jax
    from jax.sharding import Mesh, NamedSharding, PartitionSpec
    from concourse import bass2jax as b2j

    b2j.install_neuronx_cc_hook()

    partition_name = (
        nc.partition_id_tensor.name if nc.partition_id_tensor else None
    )
    in_names = []
    out_names = []
    out_avals = []
    for alloc in nc.m.functions[0].allocations:
        if not isinstance(alloc, mybir.MemoryLocationSet):
            continue
        name = alloc.memorylocations[0].name
        if alloc.kind == "ExternalInput":
            if name != partition_name:
                in_names.append(name)
        elif alloc.kind == "ExternalOutput":
            out_names.append(name)
            out_avals.append(
                jax.core.ShapedArray(
                    tuple(alloc.tensor_shape), mybir.dt.np(alloc.dtype)
                )
            )
    n_params = len(in_names)
    all_names = in_names + out_names
    if partition_name is not None:
        all_names = all_names + [partition_name]

    def _body(*args):
        operands = list(args)
        if partition_name is not None:
            operands.append(b2j.partition_id_tensor())
        outs = b2j._bass_exec_p.bind(
            *operands,
            out_avals=tuple(out_avals),
            in_names=tuple(all_names),
            out_names=tuple(out_names),
            lowering_input_output_aliases=(),
            sim_require_finite=True,
            sim_require_nnan=True,
            nc=nc,
        )
        return tuple(outs)

    devices = jax.devices()[:NCORES]
    mesh = Mesh(np.asarray(devices), ("core",))
    spec = PartitionSpec("core")
    n_outs = len(out_names)
    # No donation: output-seed buffers stay device-resident and are reused
    # across calls, so the steady-state loop never touches host memory
    # (each host->device transfer through the tunnel costs ~80ms latency).
    sharded = jax.jit(
        b2j.shard_map(
            _body,
            mesh=mesh,
            in_specs=(spec,) * (n_params + n_outs),
            out_specs=(spec,) * n_outs,
            check_rep=False,
        ),
        keep_unused=True,
    )
    return {
        "nc": nc,
        "fn": sharded,
        "in_names": in_names,
        "out_names": out_names,
        "out_avals": out_avals,
        "mesh": mesh,
        "sharding": NamedSharding(mesh, spec),
        "dev_inputs": None,
        "dev_zouts": None,
        "compiled": None,
    }


def _get_exec(n):
    if n in _EXEC:
        return _EXEC[n]
    state = make_state(build_program(n))
    _EXEC[n] = state
    return state


def _execute(state, cat_inputs=None):
    import jax

    if cat_inputs is not None:
        state["dev_inputs"] = [
            jax.device_put(cat_inputs[name], state["sharding"])
            for name in state["in_names"]
        ]
        state["dev_zouts"] = [
            jax.device_put(
                np.zeros((NCORES * a.shape[0], *a.shape[1:]), a.dtype),
                state["sharding"],
            )
            for a in state["out_avals"]
        ]
    args = [*state["dev_inputs"], *state["dev_zouts"]]
    if state["compiled"] is None:
        # AOT-compile so steady-state calls skip the jit dispatch machinery
        state["compiled"] = state["fn"].lower(*args).compile()
    outs = state["compiled"](*args)
    return [np.asarray(o) for o in outs]


def kernel(**inputs):
    n = int(np.asarray(inputs["adj"]).shape[1])
    state = _get_exec(n)
    cat = prep_concat_inputs(inputs, n)
    outs = _execute(state, cat)
    # out tensor: [NCORES*2, R] -> per-core [2, R]
    R = n // NCORES
    o = outs[0].reshape(NCORES, 2, R)
    full = np.empty((1, 2 * n), np.float32)
    for c in range(NCORES):
        full[0, c * R : (c + 1) * R] = o[c, 0]
        full[0, n + c * R : n + (c + 1) * R] = o[c, 1]
    return full


def bench(n=16384, iters=5, chain=1024):
    """Steady-state per-execution wall-clock time of the compiled
    executable, inputs already device-resident.

    Each round enqueues `chain` back-to-back executions and blocks once at
    the end, so the per-round wall time is chain * t_exec plus a single
    host<->device sync latency; total/chain is a (slightly conservative)
    per-execution time. This amortizes the tunnel's ~80ms blocking-sync
    round-trip latency, which would otherwise swamp the ~1ms kernel.
    """
    state = _EXEC.get(n)
    assert state is not None and state["dev_inputs"] is not None, (
        "call kernel() first"
    )
    fn = state["compiled"]
    args = [*state["dev_inputs"], *state["dev_zouts"]]
    # warm-up round
    outs = fn(*args)
    for o in outs:
        o.block_until_ready()
    times = []
    for _ in range(iters):
        t0 = time.perf_counter()
        for _ in range(chain):
            outs = fn(*args)
        for o in outs:
            o.block_until_ready()
        times.append((time.perf_counter() - t0) / chain)
    return min(times), times


# revision 30
# speedup vs baseline: 1.0820x; 1.0820x over previous
"""DGI (3-layer GCN encoder x2 + bilinear discriminator) Trainium2 Bass kernel.

Strategy (8 NeuronCores, 1D row-parallel over nodes):
  - Each core owns a 2048-row block of the 16384-node graph.
  - Layer 1 uses associativity: adj @ (X W1) == (adj @ X) W1, so the big
    n^2 matmul runs at width 64 per encoder instead of 264 (4.1x less PE
    work), followed by a tiny [64 x 264] projection fused into the drain.
  - adj is stored twice, pre-transposed + tiled on the host:
      A8  = e4m3(adj*n - 0.5)  (fp8, mean-centered)  -> layers 1 and 3
      A16 = fp16(adj*n)                               -> layer 2
    fp8 halves the HBM traffic of the DMA-bound layers 1/3; centering at
    the distribution mean keeps the quantization noise ~5x below raw fp8
    (verified numerically: matches the fp16-only error). The 0.5-mean term
    is restored exactly via column sums of the moving operand: for L1 it
    is folded into the bias on the host; for L3 a 0.5-weighted
    ones-matmul over p3 computes it on device (~3us).
  - Layers 1/3 run "flipped" (adj slab is the moving operand, activations
    stationary) so the PE emits transposed outputs directly, avoiding
    per-row-chunk PE transposes. Layer 2 (width 328) runs in normal
    orientation (psum [rows, 328]) which is cheaper than flipping.
  - Activations p = full [n, d] fp16 tensors in SBUF, both encoders packed
    side by side. After each layer, the local block is projected by the
    next W and AllGather'ed across the 8 cores in two row-halves that
    overlap compute.
  - Readout: node-sum of h3 (enc1) via DVE free-dim reduce on h3T,
    AllGather + local reduce, sigmoid -> c; cw = wd @ c; scores via PE
    matvec on h3T.
"""

import sys
import time

import numpy as np

sys.path.insert(0, "/opt/trn_rl_repo")

import concourse.bass as bass  # noqa: E402
import concourse.mybir as mybir  # noqa: E402
import concourse.tile as tile  # noqa: E402
from concourse import bacc  # noqa: E402

P = 128
NCORES = 8
ADJ_FRAC = 1  # diagnostic: stride over adj slabs (1 = all, 2 = half, ...)
D0, D1, D2, D3 = 64, 264, 164, 64
NS1, NS2 = 3, 2  # 128-subtiles of the (padded) dims 264 -> 3, 164 -> 2
SCALE = 16384.0
KO = 4  # k-tiles per slab DMA
GSZ = 4  # row-chunks per m-group
DT8 = mybir.dt.float8e4
DT16 = mybir.dt.float16
DT32 = mybir.dt.float32
AF = mybir.ActivationFunctionType
ALU = mybir.AluOpType


def _params(n):
    R = n // NCORES
    return dict(
        R=R,
        RC=R // P,
        KT=n // P,
        NG=(R // P) // GSZ,
        KB=(n // P) // KO,
        CH=max(1, R // (2 * P)),  # k-tiles per resident p chunk (half a rank)
        SCW=min(512, R),  # score output chunk
    )


def build_program(n=16384):
    pr = _params(n)
    R, RC, KT, NG, KB, CH, SCW = (
        pr["R"],
        pr["RC"],
        pr["KT"],
        pr["NG"],
        pr["KB"],
        pr["CH"],
        pr["SCW"],
    )
    NPC = KT // CH
    W = GSZ * P
    RH = R // 2
    RC2 = RC // 2

    nc = bacc.Bacc(
        "TRN2", target_bir_lowering=False, debug=False, num_devices=NCORES
    )

    # adjT pre-tiled on host: [NG, KB, P, KO, W]; each slab DMA is a fully
    # contiguous block (2KB-per-partition descriptors for both dtypes).
    A8 = nc.dram_tensor("adjT8", [NG, KB, P, KO, W], DT8, kind="ExternalInput").ap()
    SQX = nc.dram_tensor("seqx", [P, KT, 2 * D0], DT16, kind="ExternalInput").ap()
    W1 = nc.dram_tensor("w1", [P, D1], DT16, kind="ExternalInput").ap()
    W2 = nc.dram_tensor("w2", [NS1 * P, D2], DT16, kind="ExternalInput").ap()
    W3 = nc.dram_tensor("w3", [NS2 * P, D3], DT16, kind="ExternalInput").ap()
    B1C = nc.dram_tensor("b1c", [P, 2 * NS1], DT32, kind="ExternalInput").ap()
    B2 = nc.dram_tensor("b2", [P, 2 * D2], DT16, kind="ExternalInput").ap()
    B3 = nc.dram_tensor("b3", [P, 1], DT32, kind="ExternalInput").ap()
    WDT = nc.dram_tensor("wdt", [P, D3], DT32, kind="ExternalInput").ap()
    IDT = nc.dram_tensor("ident", [P, P], DT16, kind="ExternalInput").ap()
    SB = nc.dram_tensor("sb", [1, 2 * R], DT32, kind="ExternalInput").ap()
    OUT = nc.dram_tensor("out", [2, R], DT32, kind="ExternalOutput").ap()

    rg = [list(range(NCORES))]

    with tile.TileContext(nc) as tc:
        with (
            tc.tile_pool(name="const", bufs=1) as cp,
            tc.tile_pool(name="p2", bufs=NPC) as pp2,
            tc.tile_pool(name="p3", bufs=NPC) as pp3,
            tc.tile_pool(name="slab8", bufs=5) as s8p,
            tc.tile_pool(name="q1", bufs=2) as q1p,
            tc.tile_pool(name="h", bufs=4) as hp,
            tc.tile_pool(name="hT", bufs=2) as htp,
            tc.tile_pool(name="misc", bufs=4) as mp_,
            tc.tile_pool(name="sc", bufs=4) as scp_,
            tc.tile_pool(name="ploc", bufs=1) as plp,
            tc.tile_pool(name="ps", bufs=8, space="PSUM") as ps,
            tc.tile_pool(name="dram", bufs=1, space="DRAM") as dram,
        ):
            # ---- constants -> SBUF
            w1t = cp.tile([P, D1], DT16, name="w1t")
            nc.sync.dma_start(w1t[:], W1[:])
            w2t = cp.tile([P, NS1, D2], DT16, name="w2t")
            nc.sync.dma_start(w2t[:], W2.rearrange("(s p) d -> p s d", p=P))
            w3t = cp.tile([P, NS2, D3], DT16, name="w3t")
            nc.sync.dma_start(w3t[:], W3.rearrange("(s p) d -> p s d", p=P))
            b1c = cp.tile([P, 2 * NS1], DT32, name="b1c")
            nc.sync.dma_start(b1c[:], B1C[:])
            b2t = cp.tile([P, 2 * D2], DT16, name="b2t")
            nc.sync.dma_start(b2t[:], B2[:])
            b3t = cp.tile([P, 1], DT32, name="b3t")
            nc.sync.dma_start(b3t[:], B3[:])
            wdtt = cp.tile([P, D3], DT32, name="wdtt")
            nc.sync.dma_start(wdtt[:], WDT[:])
            idt = cp.tile([P, P], DT16, name="idt")
            nc.sync.dma_start(idt[:], IDT[:])
            halfs = cp.tile([P, 1], DT16, name="halfs")
            nc.vector.memset(halfs[:], 0.5)
            onesrow = cp.tile([1, P], DT16, name="onesrow")
            nc.vector.memset(onesrow[:], 1.0)
            # X (both encoders side by side), resident; chunked DMAs so L1
            # can start after the first chunk lands
            seqx = cp.tile([P, KT, 2 * D0], DT16, name="seqx")
            NXC = 8
            for s in range(NXC):
                lo = s * (KT // NXC)
                hi = (s + 1) * (KT // NXC)
                nc.gpsimd.dma_start(seqx[:, lo:hi, :], SQX[:, lo:hi, :])

            # ---- DRAM bounce buffers for collectives (split in row-halves
            # so each AllGather overlaps the next compute phase)
            # partition-major payload layout: store is a straight SBUF->DRAM
            # copy and every gather load is fully contiguous per partition
            # (5.25KB/2KB descriptor lines instead of 656B/256B row slices)
            p2l = [
                dram.tile([P, RC2 * 2 * D2], DT16, name=f"p2l{h}") for h in range(2)
            ]
            p2f = [
                dram.tile(
                    [NCORES * P, RC2 * 2 * D2],
                    DT16,
                    name=f"p2f{h}",
                    addr_space="Shared",
                )
                for h in range(2)
            ]
            p3l = [
                dram.tile([P, RC2 * 2 * D3], DT16, name=f"p3l{h}") for h in range(2)
            ]
            p3f = [
                dram.tile(
                    [NCORES * P, RC2 * 2 * D3],
                    DT16,
                    name=f"p3f{h}",
                    addr_space="Shared",
                )
                for h in range(2)
            ]
            ssi = dram.tile([64, 1], DT32, name="ssi")
            ssg = dram.tile([64 * NCORES, 1], DT32, name="ssg", addr_space="Shared")
            s2i = dram.tile([1, 2 * D2], DT32, name="s2i")
            s2g = dram.tile([1, 2 * D2], DT32, name="s2g", addr_space="Shared")

            # kb visit order for L2/L3: first-half chunks (even) before
            # second-half, so a layer can start while the second AllGather
            # half is in flight. Valid because KO <= CH.
            if KO <= CH:
                kb_order = [j for j in range(KB) if ((j * KO) // CH) % 2 == 0]
                kb_order += [j for j in range(KB) if ((j * KO) // CH) % 2 == 1]
            else:
                kb_order = list(range(KB))

            # ---- p staging: p_next = h @ W (local rows), per row-half; the
            # first half's matmuls + AllGather are issued mid-layer.
            # colsum_ps: optional [1, 2*d_next] psum accumulating
            # 0.5 * sum over this half's local rows of p_next (for the fp8
            # centering correction of the next layer).
            def p_stage_half(
                hT, wt, ns, d_next, ploc_bufs, pf_bufs, tagix, h, colsum_ps=None
            ):
                ploc = plp.tile(
                    [P, RC2, 2 * d_next], DT16, tag="ploc", name=f"pl{tagix}_{h}"
                )
                for rcl in range(RC2):
                    rc = h * RC2 + rcl
                    for e in range(2):
                        pq = ps.tile(
                            [P, d_next], DT32, tag="ps", name=f"pq{tagix}_{e}_{rc}"
                        )
                        for ds in range(ns):
                            nc.tensor.matmul(
                                pq[:],
                                hT[e][:, ds, rc * P : (rc + 1) * P],
                                wt[:, ds, :],
                                start=(ds == 0),
                                stop=(ds == ns - 1),
                            )
                        nc.scalar.mul(
                            ploc[:, rcl, e * d_next : (e + 1) * d_next],
                            pq[:],
                            1.0 / SCALE,
                        )
                if colsum_ps is not None:
                    csp = ps.tile(
                        [1, 2 * d_next], DT32, tag="ps", name=f"cs{tagix}_{h}"
                    )
                    for rcl in range(RC2):
                        nc.tensor.matmul(
                            csp[:],
                            halfs[:],
                            ploc[:, rcl, :],
                            start=(rcl == 0),
                            stop=(rcl == RC2 - 1),
                        )
                    # drain to SBUF immediately so the bank recycles
                    nc.vector.tensor_copy(colsum_ps[:], csp[:])
                nc.sync.dma_start(
                    ploc_bufs[h][:].rearrange("p (rc d) -> p rc d", rc=RC2), ploc[:]
                )
                nc.gpsimd.collective_compute(
                    "AllGather",
                    ALU.bypass,
                    replica_groups=rg,
                    ins=[ploc_bufs[h].opt()],
                    outs=[pf_bufs[h].opt()],
                )

            def make_pnext(pool, d_next, tagix):
                # chunk c covers k-tiles [c*CH, (c+1)*CH) = rank c//2, half c%2
                return [
                    pool.tile(
                        [P, CH, 2 * d_next], DT16, tag="p", name=f"p{tagix}c{c}"
                    )
                    for c in range(NPC)
                ]

            def p_loads_half(newp, pf_bufs, h):
                # gpsimd (SWDGE) ring so a slot-wait here never stalls the
                # sync/scalar rings that stream adjT slabs. CH == RC2, so
                # rank r's partition-major block IS chunk (2r + h) verbatim.
                for c in range(h, NPC, 2):
                    rank = c // 2
                    nc.gpsimd.dma_start(
                        newp[c][:],
                        pf_bufs[h][:][rank * P : (rank + 1) * P, :].rearrange(
                            "p (ko d) -> p ko d", ko=CH
                        ),
                    )

            # =========== Layer 1 (flipped, fp8-centered adj moving) =========
            # psum[j, r] = sum_k X[k, j] * adjc[k, r]  -> n*(adj@X).T - 0.5*s1
            # drain: h1T[ds] = relu(W1[:,ds].T @ q1T + b1c)  (b1c holds the
            # +0.5*W1.T@colsum(X) correction + n*b1, folded on the host)
            hT1 = [
                htp.tile([P, NS1, R], DT16, tag="hT", name=f"h1T{e}") for e in range(2)
            ]
            for e in range(2):
                nc.vector.memset(hT1[e][:, NS1 - 1, :], 0.0)

            p2c = make_pnext(pp2, D2, 2)
            s2hs = [
                mp_.tile([1, 2 * D2], DT32, tag="s2", name=f"s2h{h}")
                for h in range(2)
            ]

            def stage2a():
                p_stage_half(hT1, w2t, NS1, D2, p2l, p2f, 2, 0, colsum_ps=s2hs[0])
                p_loads_half(p2c, p2f, 0)

            kbs1 = list(range(KB))[::ADJ_FRAC]
            for g in range(NG):
                ps1 = ps.tile([P, W], DT32, tag="ps", name=f"q1_{g}")
                for ki, kb in enumerate(kbs1):
                    slab = s8p.tile([P, KO, W], DT8, tag="slab8", name=f"sl1_{g}_{kb}")
                    eng = nc.sync if ki % 2 == 0 else nc.scalar
                    eng.dma_start(slab[:], A8[g, kb])
                    for ko in range(KO):
                        kt = kb * KO + ko
                        nc.tensor.matmul(
                            ps1[:],
                            seqx[:, kt, :],
                            slab[:, ko, :],
                            start=(ki == 0 and ko == 0),
                            stop=(ki == len(kbs1) - 1 and ko == KO - 1),
                        )
                # drain group g
                q1sb = q1p.tile([P, W], DT16, tag="q1", name=f"q1sb_{g}")
                nc.vector.tensor_copy(q1sb[:], ps1[:])
                for e in range(2):
                    for ds in range(NS1):
                        cs = min(P, D1 - ds * P)
                        pA = ps.tile([cs, W], DT32, tag="ps", name=f"hA{g}_{e}_{ds}")
                        nc.tensor.matmul(
                            pA[:],
                            w1t[e * 64 : (e + 1) * 64, ds * P : ds * P + cs],
                            q1sb[e * 64 : (e + 1) * 64, :],
                            start=True,
                            stop=True,
                        )
                        nc.scalar.activation(
                            hT1[e][0:cs, ds, g * W : (g + 1) * W],
                            pA[:],
                            AF.Relu,
                            bias=b1c[0:cs, e * NS1 + ds : e * NS1 + ds + 1],
                        )
                if g == NG // 2 - 1:
                    stage2a()

            p_stage_half(hT1, w2t, NS1, D2, p2l, p2f, 2, 1, colsum_ps=s2hs[1])
            p_loads_half(p2c, p2f, 1)
            # combine local colsum halves and AllReduce across cores; the
            # result is consumed ~70us later (L2 group 0 drain), hiding the
            # collective latency under the first group's accumulation.
            s2sb = mp_.tile([1, 2 * D2], DT32, tag="s2", name="s2sb")
            nc.vector.tensor_tensor(s2sb[:], s2hs[0][:], s2hs[1][:], ALU.add)
            nc.scalar.dma_start(s2i[:], s2sb[:])
            nc.gpsimd.collective_compute(
                "AllReduce",
                ALU.add,
                replica_groups=rg,
                ins=[s2i.opt()],
                outs=[s2g.opt()],
            )

            # =========== Layer 2 (normal, fp8-centered adj stationary) ======
            # q2[r, j] = psum + 0.5*colsum(p2)[j]; the correction (at scale
            # S^0, since adj_s = 0.5 + adj_c at scale S) plus n*b2 forms the
            # broadcast bias tile C2 applied in the drain.
            hT2 = [
                htp.tile([P, NS2, R], DT16, tag="hT", name=f"h2T{e}") for e in range(2)
            ]
            for e in range(2):
                nc.vector.memset(hT2[e][:, NS2 - 1, :], 0.0)

            c2t = cp.tile([P, 2 * D2], DT16, name="c2t")

            def post2(rc, q2):
                r0 = rc * P
                h = hp.tile([P, 2 * D2], DT16, tag="h", name=f"h2_{rc}")
                nc.vector.tensor_tensor(h[:], q2[:], c2t[:], ALU.add)
                nc.scalar.activation(h[:], h[:], AF.Relu)
                for e in range(2):
                    for ds in range(NS2):
                        lo = e * D2 + ds * P
                        csz = min(P, D2 - ds * P)
                        tp = ps.tile([csz, P], DT16, tag="ps", name=f"t2_{rc}_{e}_{ds}")
                        nc.tensor.transpose(tp[:], h[:, lo : lo + csz], idt[:])
                        nc.vector.tensor_copy(hT2[e][0:csz, ds, r0 : r0 + P], tp[:])

            p3c = make_pnext(pp3, D3, 3)

            def stage3a():
                p_stage_half(hT2, w3t, NS2, D3, p3l, p3f, 3, 0)
                p_loads_half(p3c, p3f, 0)

            for g in range(NG):
                q2s = [
                    ps.tile([P, 2 * D2], DT32, tag="ps", name=f"q2_{g}_{mc}")
                    for mc in range(GSZ)
                ]
                kbs2 = kb_order[::ADJ_FRAC]
                for ki, kb in enumerate(kbs2):
                    slab = s8p.tile([P, KO, W], DT8, tag="slab8", name=f"sl2_{g}_{kb}")
                    eng = nc.sync if ki % 2 == 0 else nc.scalar
                    eng.dma_start(slab[:], A8[g, kb])
                    for ko in range(KO):
                        k = kb * KO + ko
                        rhs_t = p2c[k // CH]
                        for mc in range(GSZ):
                            nc.tensor.matmul(
                                q2s[mc][:],
                                slab[:, ko, mc * P : (mc + 1) * P],
                                rhs_t[:, k % CH, :],
                                start=(ki == 0 and ko == 0),
                                stop=(ki == len(kbs2) - 1 and ko == KO - 1),
                            )
                if g == 0:
                    # build C2 = bcast(0.5*colsum(p2)) + n*b2 once the
                    # AllReduce result is back (hidden under g0 accumulation)
                    s2r32 = mp_.tile([1, 2 * D2], DT32, tag="s2", name="s2r32")
                    nc.gpsimd.dma_start(s2r32[:], s2g[:])
                    s2r16 = mp_.tile([1, 2 * D2], DT16, tag="s2", name="s2r16")
                    nc.vector.tensor_copy(s2r16[:], s2r32[:])
                    psC2 = ps.tile([P, 2 * D2], DT32, tag="ps", name="psC2")
                    nc.tensor.matmul(
                        psC2[:], onesrow[:], s2r16[:], start=True, stop=True
                    )
                    nc.vector.tensor_tensor(c2t[:], psC2[:], b2t[:], ALU.add)
                for mc in range(GSZ):
                    post2(g * GSZ + mc, q2s[mc])
                if g == NG // 2 - 1:
                    stage3a()

            p_stage_half(hT2, w3t, NS2, D3, p3l, p3f, 3, 1)
            p_loads_half(p3c, p3f, 1)

            # =========== Layer 3 (flipped, fp8-centered adj moving) =========
            # psum[j, r] = sum_k p3[k, j] * adjc[k, r]; bias3 = 0.5*colsum(p3)
            # + n*b3 restores the centering term, fused into the relu drain.
            # The 0.5*colsum runs as ones(0.5)-matmuls over the gathered p3
            # chunks, split in halves so the second half can wait for its
            # AllGather while L3's group 0 accumulates.
            h3T = htp.tile([P, R], DT16, tag="hT", name="h3Tcat")
            s3ps = [ps.tile([P, 1], DT32, tag="ps", name=f"s3_{h}") for h in range(2)]

            def s3_half(h):
                idx = [(c, ch) for c in range(h, NPC, 2) for ch in range(CH)]
                for i, (c, ch) in enumerate(idx):
                    nc.tensor.matmul(
                        s3ps[h][:],
                        p3c[c][:, ch, :],
                        halfs[:],
                        start=(i == 0),
                        stop=(i == len(idx) - 1),
                    )

            s3_half(0)
            s3sb = mp_.tile([P, 1], DT32, tag="misc", name="s3sb")
            nc.vector.tensor_copy(s3sb[:], s3ps[0][:])
            bias3 = mp_.tile([P, 1], DT32, tag="misc", name="bias3")

            kbs3 = kb_order[::ADJ_FRAC]
            for g in range(NG):
                q3 = ps.tile([P, W], DT32, tag="ps", name=f"q3_{g}")
                for ki, kb in enumerate(kbs3):
                    slab = s8p.tile([P, KO, W], DT8, tag="slab8", name=f"sl3_{g}_{kb}")
                    eng = (nc.sync, nc.scalar, nc.gpsimd)[ki % 3]
                    eng.dma_start(slab[:], A8[g, kb])
                    for ko in range(KO):
                        k = kb * KO + ko
                        nc.tensor.matmul(
                            q3[:],
                            p3c[k // CH][:, k % CH, :],
                            slab[:, ko, :],
                            start=(ki == 0 and ko == 0),
                            stop=(ki == len(kbs3) - 1 and ko == KO - 1),
                        )
                if g == 0:
                    # second-half colsum + bias once its chunks have landed
                    s3_half(1)
                    nc.vector.tensor_tensor(bias3[:], s3ps[1][:], s3sb[:], ALU.add)
                    nc.vector.tensor_tensor(bias3[:], bias3[:], b3t[:], ALU.add)
                nc.scalar.activation(
                    h3T[:, g * W : (g + 1) * W],
                    q3[:],
                    AF.Relu,
                    bias=bias3[:],
                )

            # ---- readout: c = sigmoid(mean_n h3_enc1); cw = wd @ c; scores
            ss = mp_.tile([P, 1], DT32, tag="misc", name="ss")
            nc.vector.reduce_sum(
                ss[0:64, :], h3T[0:64, :], axis=mybir.AxisListType.X
            )
            nc.sync.dma_start(ssi[:], ss[0:64, :])
            nc.gpsimd.collective_compute(
                "AllGather",
                ALU.bypass,
                replica_groups=rg,
                ins=[ssi.opt()],
                outs=[ssg.opt()],
            )
            cin = mp_.tile([64, NCORES], DT32, tag="misc", name="cin")
            nc.sync.dma_start(
                cin[:], ssg[:].rearrange("(c p) one -> p (c one)", p=64)
            )
            cin2 = mp_.tile([64, 1], DT32, tag="misc", name="cin2")
            nc.vector.reduce_sum(cin2[:], cin[:], axis=mybir.AxisListType.X)
            ccol = mp_.tile([P, 1], DT32, tag="misc", name="ccol")
            nc.vector.memset(ccol[:], 0.0)
            nc.scalar.activation(
                ccol[0:64, :], cin2[:], AF.Sigmoid, scale=1.0 / (SCALE * n)
            )
            cwps = ps.tile([64, 1], DT32, tag="ps", name="cwps")
            nc.tensor.matmul(cwps[:], wdtt[:], ccol[:], start=True, stop=True)
            # two masked copies of cw: cwa selects enc1 partitions, cwb enc2
            cw16 = [
                mp_.tile([P, 1], DT16, tag="misc", name=f"cw16_{e}") for e in range(2)
            ]
            for e in range(2):
                nc.vector.memset(cw16[e][:], 0.0)
                nc.vector.tensor_copy(cw16[e][e * D3 : (e + 1) * D3, :], cwps[:])
            # score epilogue: all matmuls issued back-to-back, per-chunk
            # scale/bias/store pipelined on dedicated pool slots
            for e in range(2):
                for j in range(R // SCW):
                    scp = ps.tile([1, SCW], DT32, tag="ps", name=f"scp{e}_{j}")
                    nc.tensor.matmul(
                        scp[:],
                        cw16[e][:],
                        h3T[:, j * SCW : (j + 1) * SCW],
                        start=True,
                        stop=True,
                    )
                    sbc = scp_.tile([1, SCW], DT32, tag="sc", name=f"sbc{e}_{j}")
                    nc.sync.dma_start(
                        sbc[:], SB[:, e * R + j * SCW : e * R + (j + 1) * SCW]
                    )
                    sct = scp_.tile([1, SCW], DT32, tag="sc", name=f"sct{e}_{j}")
                    nc.scalar.mul(sct[:], scp[:], 1.0 / SCALE)
                    ot = scp_.tile([1, SCW], DT32, tag="sc", name=f"ot{e}_{j}")
                    nc.vector.tensor_tensor(ot[:], sct[:], sbc[:], ALU.add)
                    nc.scalar.dma_start(OUT[e : e + 1, j * SCW : (j + 1) * SCW], ot[:])

    nc.compile()
    return nc


# ---------------------------------------------------------------------------
# host-side input prep


def _blocked_transpose(a, B=512):
    n, m = a.shape
    out = np.empty((m, n), a.dtype)
    for i in range(0, n, B):
        for j in range(0, m, B):
            out[j : j + B, i : i + B] = a[i : i + B, j : j + B].T
    return out


def _tile_adjT(aT, NG, KB, W):
    """[n(k), n(r)] -> [NCORES*NG, KB, P, KO, W] contiguous slabs."""
    n = aT.shape[0]
    R = NG * W
    out = np.empty((NCORES * NG, KB, P, KO, W), aT.dtype)
    for c in range(NCORES):
        blk = np.ascontiguousarray(aT[:, c * R : (c + 1) * R])
        t = blk.reshape(KB, KO, P, NG, W).transpose(3, 0, 2, 1, 4)
        out[c * NG : (c + 1) * NG] = t
    return out


def prep_concat_inputs(inputs, n):
    import ml_dtypes

    R = n // NCORES
    pr = _params(n)
    KT, NG, KB = pr["KT"], pr["NG"], pr["KB"]
    W = GSZ * P

    adj = np.asarray(inputs["adj"], np.float32)[0]
    seq1 = np.asarray(inputs["seq1"], np.float32)[0]
    seq2 = np.asarray(inputs["seq2"], np.float32)[0]
    w1 = np.asarray(inputs["w1"], np.float32)
    w2 = np.asarray(inputs["w2"], np.float32)
    w3 = np.asarray(inputs["w3"], np.float32)
    b1 = np.asarray(inputs["b1"], np.float32)
    b2 = np.asarray(inputs["b2"], np.float32)
    b3 = np.asarray(inputs["b3"], np.float32)
    wd = np.asarray(inputs["wd"], np.float32)
    bd = np.float32(np.asarray(inputs["bd"]))
    sb1 = np.asarray(inputs["samp_bias1"], np.float32)[0]
    sb2 = np.asarray(inputs["samp_bias2"], np.float32)[0]

    S = np.float32(SCALE)

    # fp8 centered adjT (all three layers)
    a8 = (adj * S - np.float32(0.5)).astype(ml_dtypes.float8_e4m3)
    a8T = _blocked_transpose(a8.view(np.uint8)).view(ml_dtypes.float8_e4m3)
    del a8
    adjT8 = _tile_adjT(a8T, NG, KB, W)
    del a8T

    # X (both encoders), [P, KT, 128] layout: [p, kt, e*64+d] = seq_e[kt*P+p, d]
    X16 = np.concatenate([seq1, seq2], axis=1).astype(np.float16)  # [n, 128]
    seqx = np.ascontiguousarray(X16.reshape(KT, P, 2 * D0).transpose(1, 0, 2))

    # b1c: per-partition bias for the L1 drain, including the centering
    # correction 0.5 * W1.T @ colsum(X) (per encoder), at scale n.
    s1 = X16.astype(np.float32).sum(axis=0)  # [128]
    w1r = w1.astype(np.float16).astype(np.float32)  # match device rounding
    b1c = np.zeros((P, 2 * NS1), np.float32)
    for e in range(2):
        corr = 0.5 * (w1r.T @ s1[e * 64 : (e + 1) * 64])  # [264]
        full = b1 * S + corr
        for ds in range(NS1):
            cs = min(P, D1 - ds * P)
            b1c[0:cs, e * NS1 + ds] = full[ds * P : ds * P + cs]

    def padz(a, shape):
        out = np.zeros(shape, np.float16)
        out[: a.shape[0], : a.shape[1]] = a
        return out

    def rep(x):
        return np.tile(np.asarray(x), (NCORES, 1))

    cat = {
        "adjT8": adjT8,
        "seqx": np.tile(seqx.reshape(P, -1), (NCORES, 1)).reshape(
            NCORES * P, KT, 2 * D0
        ),
        # W1 stacked twice along partitions so each encoder's drain matmul
        # reads lhsT and rhs at the same base partition (0 or 64)
        "w1": rep(np.concatenate([w1, w1], axis=0).astype(np.float16)),
        "w2": rep(padz(w2, (NS1 * P, D2))),
        "w3": rep(padz(w3, (NS2 * P, D3))),
        "b1c": rep(b1c),
        "b2": rep(
            np.tile(
                np.concatenate([b2, b2]).astype(np.float32) * S,
                (P, 1),
            ).astype(np.float16)
        ),
        "b3": rep(
            np.concatenate([b3, b3]).astype(np.float32)[:, None] * S
        ),
        "wdt": rep(padz(wd.T, (P, D3)).astype(np.float32)),
        "ident": rep(np.eye(P, dtype=np.float16)),
        "sb": np.concatenate(
            [
                np.concatenate(
                    [sb1[c * R : (c + 1) * R] + bd, sb2[c * R : (c + 1) * R] + bd]
                )[None, :]
                for c in range(NCORES)
            ],
            axis=0,
        ).astype(np.float32),
    }
    return cat


# ---------------------------------------------------------------------------
# cached PJRT executor (compile once, run many)

_EXEC = {}


def make_state(nc):
    """Build a cached shard_map executable for a compiled Bass program."""
    import jax
    from jax.sharding import Mesh, NamedSharding, PartitionSpec
    from concourse import bass2jax as b2j

    b2j.install_neuronx_cc_hook()

    partition_name = (
        nc.partition_id_tensor.name if nc.partition_id_tensor else None
    )
    in_names = []
    out_names = []
    out_avals = []
    for alloc in nc.m.functions[0].allocations:
        if not isinstance(alloc, mybir.MemoryLocationSet):
            continue
        name = alloc.memorylocations[0].name
        if alloc.kind == "ExternalInput":
            if name != partition_name:
                in_names.append(name)
        elif alloc.kind == "ExternalOutput":
            out_names.append(name)
            out_avals.append(
                jax.core.ShapedArray(
                    tuple(alloc.tensor_shape), mybir.dt.np(alloc.dtype)
                )
            )
    n_params = len(in_names)
    all_names = in_names + out_names
    if partition_name is not None:
        all_names = all_names + [partition_name]

    def _body(*args):
        operands = list(args)
        if partition_name is not None:
            operands.append(b2j.partition_id_tensor())
        outs = b2j._bass_exec_p.bind(
            *operands,
            out_avals=tuple(out_avals),
            in_names=tuple(all_names),
            out_names=tuple(out_names),
            lowering_input_output_aliases=(),
            sim_require_finite=True,
            sim_require_nnan=True,
            nc=nc,
        )
        return tuple(outs)

    devices = jax.devices()[:NCORES]
    mesh = Mesh(np.asarray(devices), ("core",))
    spec = PartitionSpec("core")
    n_outs = len(out_names)
    # No donation: output-seed buffers stay device-resident and are reused
    # across calls, so the steady-state loop never touches host memory
    # (each host->device transfer through the tunnel costs ~80ms latency).
    sharded = jax.jit(
        b2j.shard_map(
            _body,
            mesh=mesh,
            in_specs=(spec,) * (n_params + n_outs),
            out_specs=(spec,) * n_outs,
            check_rep=False,
        ),
        keep_unused=True,
    )
    return {
        "nc": nc,
        "fn": sharded,
        "in_names": in_names,
        "out_names": out_names,
        "out_avals": out_avals,
        "mesh": mesh,
        "sharding": NamedSharding(mesh, spec),
        "dev_inputs": None,
        "dev_zouts": None,
        "compiled": None,
    }


def _get_exec(n):
    if n in _EXEC:
        return _EXEC[n]
    state = make_state(build_program(n))
    _EXEC[n] = state
    return state


def _execute(state, cat_inputs=None):
    import jax

    if cat_inputs is not None:
        state["dev_inputs"] = [
            jax.device_put(cat_inputs[name], state["sharding"])
            for name in state["in_names"]
        ]
        state["dev_zouts"] = [
            jax.device_put(
                np.zeros((NCORES * a.shape[0], *a.shape[1:]), a.dtype),
                state["sharding"],
            )
            for a in state["out_avals"]
        ]
    args = [*state["dev_inputs"], *state["dev_zouts"]]
    if state["compiled"] is None:
        # AOT-compile so steady-state calls skip the jit dispatch machinery
        state["compiled"] = state["fn"].lower(*args).compile()
    outs = state["compiled"](*args)
    return [np.asarray(o) for o in outs]


def kernel(**inputs):
    n = int(np.asarray(inputs["adj"]).shape[1])
    state = _get_exec(n)
    cat = prep_concat_inputs(inputs, n)
    outs = _execute(state, cat)
    # out tensor: [NCORES*2, R] -> per-core [2, R]
    R = n // NCORES
    o = outs[0].reshape(NCORES, 2, R)
    full = np.empty((1, 2 * n), np.float32)
    for c in range(NCORES):
        full[0, c * R : (c + 1) * R] = o[c, 0]
        full[0, n + c * R : n + (c + 1) * R] = o[c, 1]
    return full


def bench(n=16384, iters=5, chain=1024):
    """Steady-state per-execution wall-clock time of the compiled
    executable, inputs already device-resident.

    Each round enqueues `chain` back-to-back executions and blocks once at
    the end, so the per-round wall time is chain * t_exec plus a single
    host<->device sync latency; total/chain is a (slightly conservative)
    per-execution time. This amortizes the tunnel's ~80ms blocking-sync
    round-trip latency, which would otherwise swamp the ~1ms kernel.
    """
    state = _EXEC.get(n)
    assert state is not None and state["dev_inputs"] is not None, (
        "call kernel() first"
    )
    fn = state["compiled"]
    args = [*state["dev_inputs"], *state["dev_zouts"]]
    # warm-up round
    outs = fn(*args)
    for o in outs:
        o.block_until_ready()
    times = []
    for _ in range(iters):
        t0 = time.perf_counter()
        for _ in range(chain):
            outs = fn(*args)
        for o in outs:
            o.block_until_ready()
        times.append((time.perf_counter() - t0) / chain)
    return min(times), times
